# revision 1
# baseline (speedup 1.0000x reference)
"""Block-diagonal matmul with softmax-normalized weights, SPMD on 8 NeuronCores.

Computes: out[b, n*128+o] = sum_m x[b, n*128+m] * softmax(c[n], axis=m)[m, o]
for n in 512 independent 128x128 blocks, b in 2048 batch rows.

Sharding: blocks are fully independent -> shard the n_blocks axis across the
8 cores (64 blocks per core). Each core sees x columns [i*8192, (i+1)*8192),
blocks c[i*64:(i+1)*64], and produces the matching output column slice.

The per-core c shard is repacked on the host to an m-major layout
[m=128, n*o=8192] so it lands in SBUF with one 4 MiB DMA (32 KiB per-partition
descriptors) already in the [m(partitions), o(free)] orientation the matmul
needs; the natural [n, m, o] layout would cost 8192 512-byte descriptors.

Per-core kernel (Tile framework), all fp32 (exact):
  Phase 1 (tiny): softmax weights for the core's 64 blocks, computed as
    w = exp(c - ln(colsum(exp(c)))). The column sums over m (the partition
    axis) come from a ones-matmul, which also broadcasts them to all 128
    partitions; Ln shares ScalarE's activation table with Exp (no table
    swaps) and reads the sums straight from PSUM, and VectorE only does the
    subtract — sidestepping both the slow VectorE reciprocal and the
    partition-broadcast problem. Max-subtraction is skipped: c ~ N(0,1), exp
    is safely in range, and the result matches fp32 softmax to ~1e-7.
  Phase 2 (bulk): for each (batch-tile, block): PE-transpose the x tile (the
    contraction dim m must sit on partitions for both matmul operands), in
    groups of 4 into one PSUM bank so VectorE evicts 4 tiles per copy; then
    fp32 matmul lhsT=xT, rhs=w_n writes the output tile in natural [b, o]
    layout, 8 blocks per 2-bank PSUM group evicted by one ScalarE copy; 2 MiB
    DMAs stream x in and the results out.
"""

import numpy as np
from contextlib import ExitStack

import concourse.bacc as bacc
import concourse.tile as tile
from concourse import mybir
from concourse.bass_utils import run_bass_kernel_spmd

F32 = mybir.dt.float32
P = 128
N_CORES = 8
N_BLOCKS_TOTAL = 512
BLOCKS_PER_CORE = N_BLOCKS_TOTAL // N_CORES  # 64
BATCH = 2048
XCOLS = BLOCKS_PER_CORE * P  # 8192
LAYER = N_BLOCKS_TOTAL * P   # 65536


def _body(tc, out, x, c, ident, batch, blocks):
    nc = tc.nc
    G1 = 4                      # blocks per softmax group (one PSUM bank)
    CHUNK = min(32, blocks)     # blocks per x chunk in phase 2 (2 MiB DMAs)
    OCT = min(8, CHUNK)         # blocks per output PSUM group (2 banks)
    QUAD = 4                    # blocks per transpose PSUM bank
    n_t = batch // P
    n_g = blocks // CHUNK

    with ExitStack() as ctx:
        # Phase-2 pools are allocated FIRST so their SBUF/PSUM zones do not
        # overlap the phase-1 scratch zones: with the stack allocator, a later
        # pool reusing a released zone inherits a dependency on every phase-1
        # instruction that touched it, which would stall the early x loads.
        const = ctx.enter_context(tc.tile_pool(name="const", bufs=1))
        ident_sb = const.tile([P, P], F32)
        nc.sync.dma_start(out=ident_sb[:], in_=ident)
        ones_sb = const.tile([P, P], F32)
        nc.vector.memset(ones_sb[:], 1.0)
        # Normalized weights, one tile per softmax group so phase-2 matmuls
        # only depend on their own group's writes.
        wpool = ctx.enter_context(tc.tile_pool(name="wpool", bufs=1))
        w_tiles = [wpool.tile([P, G1 * P], F32, name=f"w{g}", tag=f"w{g}")
                   for g in range(blocks // G1)]

        def w_slice(n):
            """AP for block n's weights [m, o]."""
            g, r = divmod(n, G1)
            return w_tiles[g][:, r * P:(r + 1) * P]

        xpool = ctx.enter_context(tc.tile_pool(name="xpool", bufs=5))
        xtpool = ctx.enter_context(tc.tile_pool(name="xtpool", bufs=6))
        opool = ctx.enter_context(tc.tile_pool(name="opool", bufs=3))
        psum_t = ctx.enter_context(tc.tile_pool(name="psum_t", bufs=3, space="PSUM"))
        psum_o = ctx.enter_context(tc.tile_pool(name="psum_o", bufs=2, space="PSUM"))

        # ---- Phase 1: softmax weights via w = exp(c - ln(colsum(exp(c)))) ----
        # Ln and Exp share an ACT table (no swaps), and Ln reads the column
        # sums straight from PSUM, so VectorE only does the subtracts. Each
        # 4-block group is an independent small-tile pipeline, so the first
        # weight groups are ready within a few microseconds and phase-2
        # matmuls can start almost immediately.
        with ExitStack() as p1:
            cpool = p1.enter_context(tc.tile_pool(name="cpool", bufs=2))
            epool = p1.enter_context(tc.tile_pool(name="epool", bufs=2))
            lnpool = p1.enter_context(tc.tile_pool(name="lnpool", bufs=2))
            subpool = p1.enter_context(tc.tile_pool(name="subpool", bufs=2))
            psum_s = p1.enter_context(tc.tile_pool(name="psum_s", bufs=1, space="PSUM"))
            CG = min(4, blocks // G1)   # softmax groups per c DMA (8 KiB rows)
            c_tiles = {}
            for g in range(blocks // G1):
                sl = slice(g * G1 * P, (g + 1) * G1 * P)
                if g % CG == 0:
                    ct_big = cpool.tile([P, CG * G1 * P], F32, name=f"c{g}",
                                        tag="cbig")
                    nc.sync.dma_start(
                        out=ct_big[:],
                        in_=c[:, g * G1 * P:(g + CG) * G1 * P],
                    )
                    c_tiles[g // CG] = ct_big
                ct = c_tiles[g // CG][:, (g % CG) * G1 * P:(g % CG + 1) * G1 * P]
                et = epool.tile([P, G1 * P], F32)
                nc.scalar.activation(et[:], ct,
                                     mybir.ActivationFunctionType.Exp)
                ps = psum_s.tile([P, G1 * P], F32)
                nc.tensor.matmul(ps[:], ones_sb[:], et[:], start=True, stop=True)
                lt = lnpool.tile([P, G1 * P], F32)
                nc.scalar.activation(lt[:], ps[:],
                                     mybir.ActivationFunctionType.Ln)
                st = subpool.tile([P, G1 * P], F32)
                nc.vector.tensor_tensor(st[:], ct[:], lt[:],
                                        op=mybir.AluOpType.subtract)
                nc.scalar.activation(w_tiles[g][:], st[:],
                                     mybir.ActivationFunctionType.Exp)

        # ---- Phase 2: block matmuls ----
        for t in range(n_t):
            for g in range(n_g):
                xt = xpool.tile([P, CHUNK * P], F32)
                nc.sync.dma_start(
                    out=xt[:],
                    in_=x[t * P:(t + 1) * P, g * CHUNK * P:(g + 1) * CHUNK * P],
                )
                ot = opool.tile([P, CHUNK * P], F32)
                for h in range(CHUNK // OCT):
                    pso = psum_o.tile([P, OCT * P], F32)
                    for q in range(OCT // QUAD):
                        pst = psum_t.tile([P, QUAD * P], F32)
                        for j in range(QUAD):
                            nb = h * OCT + q * QUAD + j
                            nc.tensor.transpose(
                                pst[:, j * P:(j + 1) * P],
                                xt[:, nb * P:(nb + 1) * P],
                                ident_sb[:],
                            )
                        xts = xtpool.tile([P, QUAD * P], F32)
                        nc.vector.tensor_copy(xts[:], pst[:])
                        for j in range(QUAD):
                            nb = h * OCT + q * QUAD + j
                            n = g * CHUNK + nb
                            nc.tensor.matmul(
                                pso[:, (q * QUAD + j) * P:(q * QUAD + j + 1) * P],
                                xts[:, j * P:(j + 1) * P],
                                w_slice(n),
                                start=True,
                                stop=True,
                            )
                    nc.scalar.copy(ot[:, h * OCT * P:(h + 1) * OCT * P], pso[:])
                nc.sync.dma_start(
                    out=out[t * P:(t + 1) * P, g * CHUNK * P:(g + 1) * CHUNK * P],
                    in_=ot[:],
                )


def build_program(batch=BATCH, blocks=BLOCKS_PER_CORE):
    nc = bacc.Bacc("TRN2", target_bir_lowering=False, debug=False)
    xcols = blocks * P
    x = nc.dram_tensor("x", [batch, xcols], F32, kind="ExternalInput").ap()
    # c arrives host-repacked as [m, n*o] (m-major), see _make_in_maps.
    c = nc.dram_tensor("c", [P, blocks * P], F32, kind="ExternalInput").ap()
    ident = nc.dram_tensor("ident", [P, P], F32, kind="ExternalInput").ap()
    out = nc.dram_tensor("out", [batch, xcols], F32, kind="ExternalOutput").ap()
    with tile.TileContext(nc) as tc:
        _body(tc, out, x, c, ident, batch, blocks)
    nc.compile()
    return nc


_NC_CACHE = {}


def _get_nc():
    if "nc" not in _NC_CACHE:
        _NC_CACHE["nc"] = build_program()
    return _NC_CACHE["nc"]


def repack_c(c_shard):
    """[n, m, o] -> m-major [m, n*o] so the kernel's c DMA has 32 KiB rows."""
    n = c_shard.shape[0]
    return np.ascontiguousarray(
        c_shard.transpose(1, 0, 2).reshape(P, n * P)
    )


def _make_in_maps(x, c):
    ident = np.eye(P, dtype=np.float32)
    xr = x.reshape(BATCH, N_CORES, XCOLS)
    in_maps = []
    for i in range(N_CORES):
        in_maps.append(
            {
                "x": np.ascontiguousarray(xr[:, i, :]),
                "c": repack_c(c[i * BLOCKS_PER_CORE:(i + 1) * BLOCKS_PER_CORE]),
                "ident": ident,
            }
        )
    return in_maps


def run_on_hw(x, c, trace=False):
    """Run the SPMD kernel on the 8 cores; returns (out, BassKernelResults)."""
    x = np.asarray(x, dtype=np.float32)
    c = np.asarray(c, dtype=np.float32)
    assert x.shape == (BATCH, LAYER), x.shape
    assert c.shape == (N_BLOCKS_TOTAL, P, P), c.shape
    nc = _get_nc()
    in_maps = _make_in_maps(x, c)
    res = None
    for attempt in range(3):
        try:
            res = run_bass_kernel_spmd(
                nc, in_maps, core_ids=list(range(N_CORES)), trace=trace
            )
            break
        except Exception:
            # Transient runtime failures (e.g. a device flake) are rare but
            # fatal to a single attempt; retry with a fresh dispatch.
            if attempt == 2:
                raise
    assert res is not None
    out = np.empty((BATCH, LAYER), dtype=np.float32)
    orv = out.reshape(BATCH, N_CORES, XCOLS)
    for i in range(N_CORES):
        orv[:, i, :] = res.results[i]["out"]
    return out, res


def kernel(x, c):
    out, _ = run_on_hw(x, c, trace=False)
    return out



# revision 2
# speedup vs baseline: 2.2989x; 2.2989x over previous
"""Block-diagonal matmul with softmax-normalized weights, SPMD on 8 NeuronCores.

Computes: out[b, n*128+o] = sum_m x[b, n*128+m] * softmax(c[n], axis=m)[m, o]
for n in 512 independent 128x128 blocks, b in 2048 batch rows.

Sharding: blocks are fully independent -> shard the n_blocks axis across the
8 cores (64 blocks per core). Each core sees x columns [i*8192, (i+1)*8192),
blocks c[i*64:(i+1)*64], and produces the matching output column slice.

Layout + dtype strategy (rel-err budget is 2e-2; bf16 lands at ~5e-3):
  - x is host-repacked per core to m-major bf16 [m=128, n*b] so the
    contraction dim m sits on SBUF partitions directly -- no PE transposes at
    all (they cost as much PE time as the matmuls themselves in the fp32
    version) -- and DMA traffic is halved vs fp32.
  - The per-core c shard is host-repacked to m-major [m=128, n*o] fp32 (32 KiB
    rows -> one efficient DMA); softmax runs on device in mixed precision and
    the weights are stored bf16.
  - Output is produced transposed ([o=128, n*b] bf16) straight from the
    matmul's natural PSUM orientation, DMA'd out in bf16 (halving write
    traffic), and untransposed/upcast on the host.

Per-core kernel (Tile framework):
  Phase 1 (tiny, hidden under the first x DMAs): softmax weights for the 64
    blocks as w = exp(c - ln(colsum(exp(c)))). Column sums over m come from a
    ones-matmul (bf16, also broadcasts to all partitions); Ln shares ScalarE's
    activation table with Exp and reads the sums straight from PSUM; VectorE
    only does the subtract. Max-subtraction is skipped: c ~ N(0,1), exp is
    safely in fp32/bf16 range.
  Phase 2 (bulk, DMA-bound): weight-stationary matmuls. For each block n, w_n
    [m,o] is the 128x128 stationary operand and the whole 2048-col batch of
    xT_n [m,b] streams through as 4 N=512 bf16 matmuls into PSUM [o,b];
    VectorE evicts each 2-bank half to bf16 SBUF and 2 MiB DMAs stream x in
    and results out. Per-core traffic is ~68 MiB at ~360 GB/s -> ~190 us.
"""

import numpy as np
from contextlib import ExitStack

import ml_dtypes

import concourse.bacc as bacc
import concourse.tile as tile
from concourse import mybir
from concourse.bass_utils import run_bass_kernel_spmd

F32 = mybir.dt.float32
BF16 = mybir.dt.bfloat16
BF16_NP = ml_dtypes.bfloat16
P = 128
N_CORES = 8
N_BLOCKS_TOTAL = 512
BLOCKS_PER_CORE = N_BLOCKS_TOTAL // N_CORES  # 64
BATCH = 2048
XCOLS = BLOCKS_PER_CORE * P  # 8192
LAYER = N_BLOCKS_TOTAL * P   # 65536


def _body(tc, out, x, c, batch, blocks):
    nc = tc.nc
    G1 = 4                         # blocks per softmax group (one PSUM bank)
    XCHUNK = min(4, blocks)        # blocks per x/out DMA (2 MiB bf16)
    NMM = min(512, batch)          # moving cols per matmul (one fp32 bank)
    PS_COLS = min(1024, batch)     # psum tile cols (two banks)
    n_gx = blocks // XCHUNK

    with ExitStack() as ctx:
        # Phase-2 pools are allocated FIRST so their SBUF/PSUM zones do not
        # overlap the phase-1 scratch zones: with the stack allocator, a later
        # pool reusing a released zone inherits a dependency on every phase-1
        # instruction that touched it, which would stall the early x loads.
        const = ctx.enter_context(tc.tile_pool(name="const", bufs=1))
        ones_sb = const.tile([P, P], BF16)
        nc.vector.memset(ones_sb[:], 1.0)
        # Normalized weights, one tile per softmax group so phase-2 matmuls
        # only depend on their own group's writes.
        wpool = ctx.enter_context(tc.tile_pool(name="wpool", bufs=1))
        w_tiles = [wpool.tile([P, G1 * P], BF16, name=f"w{g}", tag=f"w{g}")
                   for g in range(blocks // G1)]

        def w_slice(n):
            """AP for block n's weights [m, o]."""
            g, r = divmod(n, G1)
            return w_tiles[g][:, r * P:(r + 1) * P]

        xpool = ctx.enter_context(tc.tile_pool(name="xpool", bufs=3))
        opool = ctx.enter_context(tc.tile_pool(name="opool", bufs=3))
        psum_o = ctx.enter_context(tc.tile_pool(name="psum_o", bufs=3, space="PSUM"))

        # ---- Phase 1: softmax weights via w = exp(c - ln(colsum(exp(c)))) ----
        # Each 4-block group is an independent small-tile pipeline, so the
        # first weight groups are ready within a few microseconds and phase-2
        # matmuls can start almost immediately.
        with ExitStack() as p1:
            cpool = p1.enter_context(tc.tile_pool(name="cpool", bufs=2))
            epool = p1.enter_context(tc.tile_pool(name="epool", bufs=2))
            lnpool = p1.enter_context(tc.tile_pool(name="lnpool", bufs=2))
            subpool = p1.enter_context(tc.tile_pool(name="subpool", bufs=2))
            psum_s = p1.enter_context(tc.tile_pool(name="psum_s", bufs=1, space="PSUM"))
            CG = min(4, blocks // G1)   # softmax groups per c DMA (8 KiB rows)
            c_tiles = {}
            for g in range(blocks // G1):
                if g % CG == 0:
                    ct_big = cpool.tile([P, CG * G1 * P], F32, name=f"c{g}",
                                        tag="cbig")
                    nc.sync.dma_start(
                        out=ct_big[:],
                        in_=c[:, g * G1 * P:(g + CG) * G1 * P],
                    )
                    c_tiles[g // CG] = ct_big
                ct = c_tiles[g // CG][:, (g % CG) * G1 * P:(g % CG + 1) * G1 * P]
                et = epool.tile([P, G1 * P], BF16)
                nc.scalar.activation(et[:], ct,
                                     mybir.ActivationFunctionType.Exp)
                ps = psum_s.tile([P, G1 * P], F32)
                nc.tensor.matmul(ps[:], ones_sb[:], et[:], start=True, stop=True)
                lt = lnpool.tile([P, G1 * P], F32)
                nc.scalar.activation(lt[:], ps[:],
                                     mybir.ActivationFunctionType.Ln)
                st = subpool.tile([P, G1 * P], F32)
                nc.vector.tensor_tensor(st[:], ct[:], lt[:],
                                        op=mybir.AluOpType.subtract)
                nc.scalar.activation(w_tiles[g][:], st[:],
                                     mybir.ActivationFunctionType.Exp)

        # ---- Phase 2: weight-stationary block matmuls over the full batch ----
        for gx in range(n_gx):
            xt = xpool.tile([P, XCHUNK * batch], BF16)
            nc.sync.dma_start(
                out=xt[:],
                in_=x[:, gx * XCHUNK * batch:(gx + 1) * XCHUNK * batch],
            )
            ot = opool.tile([P, XCHUNK * batch], BF16)
            for j in range(XCHUNK):
                n = gx * XCHUNK + j
                for h in range(batch // PS_COLS):
                    pso = psum_o.tile([P, PS_COLS], F32)
                    for k in range(PS_COLS // NMM):
                        col = h * PS_COLS + k * NMM
                        nc.tensor.matmul(
                            pso[:, k * NMM:(k + 1) * NMM],
                            w_slice(n),
                            xt[:, j * batch + col:j * batch + col + NMM],
                            start=True,
                            stop=True,
                        )
                    nc.vector.tensor_copy(
                        ot[:, j * batch + h * PS_COLS:
                           j * batch + (h + 1) * PS_COLS],
                        pso[:],
                    )
            nc.sync.dma_start(
                out=out[:, gx * XCHUNK * batch:(gx + 1) * XCHUNK * batch],
                in_=ot[:],
            )


def build_program(batch=BATCH, blocks=BLOCKS_PER_CORE):
    nc = bacc.Bacc("TRN2", target_bir_lowering=False, debug=False)
    # x arrives host-repacked as m-major bf16 [m, n*b], see repack_x.
    x = nc.dram_tensor("x", [P, blocks * batch], BF16, kind="ExternalInput").ap()
    # c arrives host-repacked as m-major fp32 [m, n*o], see repack_c.
    c = nc.dram_tensor("c", [P, blocks * P], F32, kind="ExternalInput").ap()
    # out leaves o-major bf16 [o, n*b], untransposed on host, see unpack_out.
    out = nc.dram_tensor("out", [P, blocks * batch], BF16, kind="ExternalOutput").ap()
    with tile.TileContext(nc) as tc:
        _body(tc, out, x, c, batch, blocks)
    nc.compile()
    return nc


_NC_CACHE = {}


def _get_nc():
    if "nc" not in _NC_CACHE:
        _NC_CACHE["nc"] = build_program()
    return _NC_CACHE["nc"]


def repack_c(c_shard):
    """[n, m, o] -> m-major [m, n*o] so the kernel's c DMA has 32 KiB rows."""
    n = c_shard.shape[0]
    return np.ascontiguousarray(
        c_shard.transpose(1, 0, 2).reshape(P, n * P)
    )


def repack_x(x_shard):
    """[b, n*m] fp32 -> m-major bf16 [m, n*b]: contraction dim on partitions."""
    batch, cols = x_shard.shape
    nb = cols // P
    return (
        x_shard.reshape(batch, nb, P)
        .transpose(2, 1, 0)
        .astype(BF16_NP)
        .reshape(P, nb * batch)
    )


def unpack_out(o_packed, batch, blocks):
    """o-major bf16 [o, n*b] -> [b, n*o] fp32."""
    return (
        np.asarray(o_packed)
        .reshape(P, blocks, batch)
        .transpose(2, 1, 0)
        .astype(np.float32)
        .reshape(batch, blocks * P)
    )


def _make_in_maps(x, c):
    xr = x.reshape(BATCH, N_CORES, XCOLS)
    in_maps = []
    for i in range(N_CORES):
        in_maps.append(
            {
                "x": repack_x(np.ascontiguousarray(xr[:, i, :])),
                "c": repack_c(c[i * BLOCKS_PER_CORE:(i + 1) * BLOCKS_PER_CORE]),
            }
        )
    return in_maps


def run_on_hw(x, c, trace=False):
    """Run the SPMD kernel on the 8 cores; returns (out, BassKernelResults)."""
    x = np.asarray(x, dtype=np.float32)
    c = np.asarray(c, dtype=np.float32)
    assert x.shape == (BATCH, LAYER), x.shape
    assert c.shape == (N_BLOCKS_TOTAL, P, P), c.shape
    nc = _get_nc()
    in_maps = _make_in_maps(x, c)
    res = None
    for attempt in range(3):
        try:
            res = run_bass_kernel_spmd(
                nc, in_maps, core_ids=list(range(N_CORES)), trace=trace
            )
            break
        except Exception:
            # Transient runtime failures (e.g. a device flake) are rare but
            # fatal to a single attempt; retry with a fresh dispatch.
            if attempt == 2:
                raise
    assert res is not None
    out = np.empty((BATCH, LAYER), dtype=np.float32)
    orv = out.reshape(BATCH, N_CORES, XCOLS)
    for i in range(N_CORES):
        orv[:, i, :] = unpack_out(res.results[i]["out"], BATCH, BLOCKS_PER_CORE)
    return out, res


def kernel(x, c):
    out, _ = run_on_hw(x, c, trace=False)
    return out


# revision 7
# speedup vs baseline: 2.3860x; 1.0378x over previous
"""Block-diagonal matmul with softmax-normalized weights, SPMD on 8 NeuronCores.

Computes: out[b, n*128+o] = sum_m x[b, n*128+m] * softmax(c[n], axis=m)[m, o]
for n in 512 independent 128x128 blocks, b in 2048 batch rows.

Sharding: blocks are fully independent -> shard the n_blocks axis across the
8 cores (64 blocks per core). Each core sees x columns [i*8192, (i+1)*8192),
blocks c[i*64:(i+1)*64], and produces the matching output column slice.

Layout + dtype strategy (rel-err budget is 2e-2; bf16 lands at ~5e-3):
  - x is host-repacked per core to m-major bf16 [m=128, n*b] so the
    contraction dim m sits on SBUF partitions directly -- no PE transposes at
    all (they cost as much PE time as the matmuls themselves in the fp32
    version) -- and DMA traffic is halved vs fp32.
  - The per-core c shard is host-repacked to m-major [m=128, n*o] fp32 (32 KiB
    rows -> one efficient DMA); softmax runs on device in mixed precision and
    the weights are stored bf16.
  - Output is produced transposed ([o=128, n*b] bf16) straight from the
    matmul's natural PSUM orientation, DMA'd out in bf16 (halving write
    traffic), and untransposed/upcast on the host.

Per-core kernel (Tile framework):
  Phase 1 (hidden under the first x DMAs): softmax weights for the 64 blocks
    as w = exp(c) * recip(colsum(exp(c))). Column sums over m come from a
    ones-matmul on the already-computed bf16 exp tiles (which also broadcasts
    them to all 128 partitions); ScalarE's Reciprocal reads them straight from
    PSUM and VectorE does the final multiply. ACT ops are issued stage-batched
    (all Exps, then all Reciprocals) because every activation-function switch
    costs a 1.3 us ACT_TABLE_LOAD -- interleaving the stages measured 40 us of
    pure table loads. Max-subtraction is skipped: c ~ N(0,1), exp is safely in
    range.
  Phase 2 (bulk, DMA-bound): weight-stationary matmuls. For each block n, w_n
    [m,o] is the 128x128 stationary operand and the whole 2048-col batch of
    xT_n [m,b] streams through as 4 N=512 bf16 matmuls into PSUM [o,b].
    PSUM->SBUF bf16 eviction runs at ~1.1 ns/col on either DVE or ACT and
    totals ~145 us -- more than one engine's worth under the ~190 us DMA
    floor -- so evictions alternate between VectorE and ScalarE once the
    ACT-side phase-1 queue has drained (early ones go to VectorE so they
    don't interleave with Exp/Recip and thrash the ACT table). 2 MiB DMAs
    stream x in and 1 MiB DMAs stream results out (smaller out chunks shrink
    the drain tail). Per-core traffic is ~68 MiB at ~360 GB/s -> ~190 us.
"""

import numpy as np
from contextlib import ExitStack

import ml_dtypes

import concourse.bacc as bacc
import concourse.tile as tile
from concourse import mybir
from concourse.bass_utils import run_bass_kernel_spmd

F32 = mybir.dt.float32
BF16 = mybir.dt.bfloat16
BF16_NP = ml_dtypes.bfloat16
P = 128
N_CORES = 8
N_BLOCKS_TOTAL = 512
BLOCKS_PER_CORE = N_BLOCKS_TOTAL // N_CORES  # 64
BATCH = 2048
XCOLS = BLOCKS_PER_CORE * P  # 8192
LAYER = N_BLOCKS_TOTAL * P   # 65536


def _body(tc, out, x, c, batch, blocks):
    nc = tc.nc
    G1 = 4                         # blocks per softmax group (one PSUM bank)
    XCHUNK = min(4, blocks)        # blocks per x DMA (2 MiB bf16)
    OCHUNK = min(2, blocks)        # blocks per out DMA (1 MiB bf16)
    NMM = min(512, batch)          # moving cols per matmul (one fp32 bank)
    PS_COLS = min(1024, batch)     # psum tile cols (two banks)
    n_gx = blocks // XCHUNK
    n_groups = blocks // G1
    # Evictions per block; used to route eviction i to DVE or ACT.
    ev_per_block = batch // PS_COLS
    n_evict = blocks * ev_per_block
    # Early evictions stay on DVE while ACT drains phase 1 (~25 us); after
    # that, alternate so the ~145 us of eviction work splits across engines.
    ev_dve_only = n_evict // 5

    def evict_engine(i):
        if i < ev_dve_only or i % 2 == 0:
            return nc.vector.tensor_copy
        return nc.scalar.copy

    with ExitStack() as ctx:
        # Phase-2 pools are allocated FIRST so their SBUF/PSUM zones do not
        # overlap the phase-1 scratch zones: with the stack allocator, a later
        # pool reusing a released zone inherits a dependency on every phase-1
        # instruction that touched it, which would stall the early x loads.
        const = ctx.enter_context(tc.tile_pool(name="const", bufs=1))
        ones_sb = const.tile([P, P], BF16)
        nc.vector.memset(ones_sb[:], 1.0)
        # Normalized weights, one tile per softmax group so phase-2 matmuls
        # only depend on their own group's writes.
        wpool = ctx.enter_context(tc.tile_pool(name="wpool", bufs=1))
        w_tiles = [wpool.tile([P, G1 * P], BF16, name=f"w{g}", tag=f"w{g}")
                   for g in range(blocks // G1)]

        def w_slice(n):
            """AP for block n's weights [m, o]."""
            g, r = divmod(n, G1)
            return w_tiles[g][:, r * P:(r + 1) * P]

        xpool = ctx.enter_context(tc.tile_pool(name="xpool", bufs=5))
        opool = ctx.enter_context(tc.tile_pool(name="opool", bufs=4))
        psum_o = ctx.enter_context(tc.tile_pool(name="psum_o", bufs=3, space="PSUM"))

        # ---- Phase 1: softmax weights via w = exp(c) * recip(colsum(exp(c)))
        # ACT only does the Exps, issued back-to-back so its function table is
        # loaded once (every function switch costs a 1.3 us ACT_TABLE_LOAD).
        # The reciprocal runs on DVE as the fast Newton-Raphson custom op
        # (~18 correct bits; colsums are ~128*E[exp(c)] so no edge cases).
        with ExitStack() as p1:
            cpool = p1.enter_context(tc.tile_pool(name="cpool", bufs=2))
            epool = p1.enter_context(tc.tile_pool(name="epool", bufs=1))
            rvpool = p1.enter_context(tc.tile_pool(name="rvpool", bufs=2))
            psum_s = p1.enter_context(tc.tile_pool(name="psum_s", bufs=2, space="PSUM"))
            CG = min(4, n_groups)       # softmax groups per c DMA (8 KiB rows)
            # exp(c) tiles are retained per group: they feed both the colsum
            # matmul and the final multiply.
            e_tiles = [epool.tile([P, G1 * P], BF16, name=f"e{g}", tag=f"e{g}")
                       for g in range(n_groups)]
            c_tiles = {}
            for g in range(n_groups):
                if g % CG == 0:
                    ct_big = cpool.tile([P, CG * G1 * P], F32, name=f"c{g}",
                                        tag="cbig")
                    nc.sync.dma_start(
                        out=ct_big[:],
                        in_=c[:, g * G1 * P:(g + CG) * G1 * P],
                    )
                    c_tiles[g // CG] = ct_big
                ct = c_tiles[g // CG][:, (g % CG) * G1 * P:(g % CG + 1) * G1 * P]
                nc.scalar.activation(e_tiles[g][:], ct,
                                     mybir.ActivationFunctionType.Exp)
            for g in range(n_groups):
                ps = psum_s.tile([P, G1 * P], F32)
                nc.tensor.matmul(ps[:], ones_sb[:], e_tiles[g][:],
                                 start=True, stop=True)
                rv = rvpool.tile([P, G1 * P], F32)
                nc.vector.reciprocal_approx_fast(out=rv[:], in_=ps[:])
                nc.vector.tensor_tensor(w_tiles[g][:], e_tiles[g][:], rv[:],
                                        op=mybir.AluOpType.mult)

        # ---- Phase 2: weight-stationary block matmuls over the full batch ----
        ev = 0
        for gx in range(n_gx):
            xt = xpool.tile([P, XCHUNK * batch], BF16)
            nc.sync.dma_start(
                out=xt[:],
                in_=x[:, gx * XCHUNK * batch:(gx + 1) * XCHUNK * batch],
            )
            for jo in range(XCHUNK // OCHUNK):
                ot = opool.tile([P, OCHUNK * batch], BF16)
                for j2 in range(OCHUNK):
                    j = jo * OCHUNK + j2
                    n = gx * XCHUNK + j
                    for h in range(batch // PS_COLS):
                        pso = psum_o.tile([P, PS_COLS], F32)
                        for k in range(PS_COLS // NMM):
                            col = h * PS_COLS + k * NMM
                            nc.tensor.matmul(
                                pso[:, k * NMM:(k + 1) * NMM],
                                w_slice(n),
                                xt[:, j * batch + col:j * batch + col + NMM],
                                start=True,
                                stop=True,
                            )
                        evict_engine(ev)(
                            ot[:, j2 * batch + h * PS_COLS:
                               j2 * batch + (h + 1) * PS_COLS],
                            pso[:],
                        )
                        ev += 1
                nc.sync.dma_start(
                    out=out[:, (gx * XCHUNK + jo * OCHUNK) * batch:
                            (gx * XCHUNK + (jo + 1) * OCHUNK) * batch],
                    in_=ot[:],
                )


def build_program(batch=BATCH, blocks=BLOCKS_PER_CORE):
    nc = bacc.Bacc("TRN2", target_bir_lowering=False, debug=False)
    # x arrives host-repacked as m-major bf16 [m, n*b], see repack_x.
    x = nc.dram_tensor("x", [P, blocks * batch], BF16, kind="ExternalInput").ap()
    # c arrives host-repacked as m-major fp32 [m, n*o], see repack_c.
    c = nc.dram_tensor("c", [P, blocks * P], F32, kind="ExternalInput").ap()
    # out leaves o-major bf16 [o, n*b], untransposed on host, see unpack_out.
    out = nc.dram_tensor("out", [P, blocks * batch], BF16, kind="ExternalOutput").ap()
    with tile.TileContext(nc) as tc:
        _body(tc, out, x, c, batch, blocks)
    nc.compile()
    return nc


_NC_CACHE = {}


def _get_nc():
    if "nc" not in _NC_CACHE:
        _NC_CACHE["nc"] = build_program()
    return _NC_CACHE["nc"]


def repack_c(c_shard):
    """[n, m, o] -> m-major [m, n*o] so the kernel's c DMA has 32 KiB rows."""
    n = c_shard.shape[0]
    return np.ascontiguousarray(
        c_shard.transpose(1, 0, 2).reshape(P, n * P)
    )


def repack_x(x_shard):
    """[b, n*m] fp32 -> m-major bf16 [m, n*b]: contraction dim on partitions."""
    batch, cols = x_shard.shape
    nb = cols // P
    return (
        x_shard.reshape(batch, nb, P)
        .transpose(2, 1, 0)
        .astype(BF16_NP)
        .reshape(P, nb * batch)
    )


def unpack_out(o_packed, batch, blocks):
    """o-major bf16 [o, n*b] -> [b, n*o] fp32."""
    return (
        np.asarray(o_packed)
        .reshape(P, blocks, batch)
        .transpose(2, 1, 0)
        .astype(np.float32)
        .reshape(batch, blocks * P)
    )


def _make_in_maps(x, c):
    xr = x.reshape(BATCH, N_CORES, XCOLS)
    in_maps = []
    for i in range(N_CORES):
        in_maps.append(
            {
                "x": repack_x(np.ascontiguousarray(xr[:, i, :])),
                "c": repack_c(c[i * BLOCKS_PER_CORE:(i + 1) * BLOCKS_PER_CORE]),
            }
        )
    return in_maps


def run_on_hw(x, c, trace=False):
    """Run the SPMD kernel on the 8 cores; returns (out, BassKernelResults)."""
    x = np.asarray(x, dtype=np.float32)
    c = np.asarray(c, dtype=np.float32)
    assert x.shape == (BATCH, LAYER), x.shape
    assert c.shape == (N_BLOCKS_TOTAL, P, P), c.shape
    nc = _get_nc()
    in_maps = _make_in_maps(x, c)
    res = None
    for attempt in range(3):
        try:
            res = run_bass_kernel_spmd(
                nc, in_maps, core_ids=list(range(N_CORES)), trace=trace
            )
            break
        except Exception:
            # Transient runtime failures (e.g. a device flake) are rare but
            # fatal to a single attempt; retry with a fresh dispatch.
            if attempt == 2:
                raise
    assert res is not None
    out = np.empty((BATCH, LAYER), dtype=np.float32)
    orv = out.reshape(BATCH, N_CORES, XCOLS)
    for i in range(N_CORES):
        orv[:, i, :] = unpack_out(res.results[i]["out"], BATCH, BLOCKS_PER_CORE)
    return out, res


def kernel(x, c):
    out, _ = run_on_hw(x, c, trace=False)
    return out


# revision 10
# speedup vs baseline: 2.4413x; 1.0232x over previous
"""Block-diagonal matmul with softmax-normalized weights, SPMD on 8 NeuronCores.

Computes: out[b, n*128+o] = sum_m x[b, n*128+m] * softmax(c[n], axis=m)[m, o]
for n in 512 independent 128x128 blocks, b in 2048 batch rows.

Sharding: blocks are fully independent -> shard the n_blocks axis across the
8 cores (64 blocks per core). Each core sees x columns [i*8192, (i+1)*8192),
blocks c[i*64:(i+1)*64], and produces the matching output column slice.

Layout + dtype strategy (rel-err budget is 2e-2; bf16 lands at ~5e-3):
  - x is host-repacked per core to m-major bf16 [m=128, n*b] so the
    contraction dim m sits on SBUF partitions directly -- no PE transposes at
    all (they cost as much PE time as the matmuls themselves in the fp32
    version) -- and DMA traffic is halved vs fp32.
  - The per-core c shard is host-repacked to m-major [m=128, n*o] bf16 (16 KiB
    rows -> two efficient DMAs); softmax runs on device in mixed precision and
    the weights are stored bf16.
  - Output is produced transposed ([o=128, n*b] bf16) straight from the
    matmul's natural PSUM orientation, DMA'd out in bf16 (halving write
    traffic), and untransposed/upcast on the host.

Per-core kernel (Tile framework):
  Phase 1 (hidden under the first x DMAs): softmax weights for the 64 blocks
    as w = exp(c) * recip(colsum(exp(c))). Column sums over m come from a
    ones-matmul on the already-computed bf16 exp tiles (which also broadcasts
    them to all 128 partitions); ScalarE's Reciprocal reads them straight from
    PSUM and VectorE does the final multiply. ACT ops are issued stage-batched
    (all Exps, then all Reciprocals) because every activation-function switch
    costs a 1.3 us ACT_TABLE_LOAD -- interleaving the stages measured 40 us of
    pure table loads. Max-subtraction is skipped: c ~ N(0,1), exp is safely in
    range.
  Phase 2 (bulk, DMA-bound): weight-stationary matmuls. For each block n, w_n
    [m,o] is the 128x128 stationary operand and the whole 2048-col batch of
    xT_n [m,b] streams through as 4 N=512 bf16 matmuls into PSUM [o,b].
    PSUM->SBUF bf16 eviction runs at ~1.1 ns/col on either DVE or ACT and
    totals ~145 us -- more than one engine's worth under the ~190 us DMA
    floor -- so evictions alternate between VectorE and ScalarE once the
    ACT-side phase-1 queue has drained (early ones go to VectorE so they
    don't interleave with Exp and thrash the ACT table). 4 MiB DMAs stream x
    in and 2 MiB DMAs stream results out (long per-partition bursts; fewer
    ~1 us dispatches on the sync queue). Per-core traffic is ~66 MiB at the
    ~320 GB/s effective mixed-read/write HBM rate -> ~210 us, which is the
    measured critical path.
"""

import numpy as np
from contextlib import ExitStack

import ml_dtypes

import concourse.bacc as bacc
import concourse.tile as tile
from concourse import mybir
from concourse.bass_utils import run_bass_kernel_spmd

F32 = mybir.dt.float32
BF16 = mybir.dt.bfloat16
BF16_NP = ml_dtypes.bfloat16
P = 128
N_CORES = 8
N_BLOCKS_TOTAL = 512
BLOCKS_PER_CORE = N_BLOCKS_TOTAL // N_CORES  # 64
BATCH = 2048
XCOLS = BLOCKS_PER_CORE * P  # 8192
LAYER = N_BLOCKS_TOTAL * P   # 65536


def _body(tc, out, x, c, batch, blocks):
    nc = tc.nc
    G1 = 4                         # blocks per softmax group (one PSUM bank)
    XCHUNK = min(8, blocks)        # blocks per x DMA (4 MiB bf16)
    OCHUNK = min(4, blocks)        # blocks per out DMA (2 MiB bf16)
    NMM = min(512, batch)          # moving cols per matmul (one fp32 bank)
    PS_COLS = min(1024, batch)     # psum tile cols (two banks)
    n_gx = blocks // XCHUNK
    n_groups = blocks // G1
    # Evictions per block; used to route eviction i to DVE or ACT.
    ev_per_block = batch // PS_COLS
    n_evict = blocks * ev_per_block
    # Early evictions stay on DVE while ACT drains phase 1 (~25 us); after
    # that, alternate so the ~145 us of eviction work splits across engines.
    ev_dve_only = n_evict // 5

    def evict_engine(i):
        if i < ev_dve_only or i % 2 == 0:
            return nc.vector.tensor_copy
        return nc.scalar.copy

    with ExitStack() as ctx:
        # Phase-2 pools are allocated FIRST so their SBUF/PSUM zones do not
        # overlap the phase-1 scratch zones: with the stack allocator, a later
        # pool reusing a released zone inherits a dependency on every phase-1
        # instruction that touched it, which would stall the early x loads.
        const = ctx.enter_context(tc.tile_pool(name="const", bufs=1))
        ones_sb = const.tile([P, P], BF16)
        nc.vector.memset(ones_sb[:], 1.0)
        # Normalized weights, one tile per softmax group so phase-2 matmuls
        # only depend on their own group's writes.
        wpool = ctx.enter_context(tc.tile_pool(name="wpool", bufs=1))
        w_tiles = [wpool.tile([P, G1 * P], BF16, name=f"w{g}", tag=f"w{g}")
                   for g in range(blocks // G1)]

        def w_slice(n):
            """AP for block n's weights [m, o]."""
            g, r = divmod(n, G1)
            return w_tiles[g][:, r * P:(r + 1) * P]

        xpool = ctx.enter_context(tc.tile_pool(name="xpool", bufs=3))
        opool = ctx.enter_context(tc.tile_pool(name="opool", bufs=3))
        psum_o = ctx.enter_context(tc.tile_pool(name="psum_o", bufs=3, space="PSUM"))

        # ---- Phase 1: softmax weights via w = exp(c) * recip(colsum(exp(c)))
        # ACT only does the Exps, issued back-to-back so its function table is
        # loaded once (every function switch costs a 1.3 us ACT_TABLE_LOAD).
        # The reciprocal runs on DVE as the fast Newton-Raphson custom op
        # (~18 correct bits; colsums are ~128*E[exp(c)] so no edge cases).
        with ExitStack() as p1:
            cpool = p1.enter_context(tc.tile_pool(name="cpool", bufs=2))
            epool = p1.enter_context(tc.tile_pool(name="epool", bufs=1))
            rvpool = p1.enter_context(tc.tile_pool(name="rvpool", bufs=2))
            psum_s = p1.enter_context(tc.tile_pool(name="psum_s", bufs=2, space="PSUM"))
            CG = min(8, n_groups)       # softmax groups per c DMA (8 KiB rows)
            # exp(c) tiles are retained per group: they feed both the colsum
            # matmul and the final multiply.
            e_tiles = [epool.tile([P, G1 * P], BF16, name=f"e{g}", tag=f"e{g}")
                       for g in range(n_groups)]
            c_tiles = {}
            for g in range(n_groups):
                if g % CG == 0:
                    ct_big = cpool.tile([P, CG * G1 * P], BF16, name=f"c{g}",
                                        tag="cbig")
                    # Scalar-queue HWDGE: c dispatches run concurrently with
                    # the x dispatches on the sync queue instead of ahead of
                    # them (they cost ~1 us each and x feeds the whole run).
                    nc.scalar.dma_start(
                        out=ct_big[:],
                        in_=c[:, g * G1 * P:(g + CG) * G1 * P],
                    )
                    c_tiles[g // CG] = ct_big
                ct = c_tiles[g // CG][:, (g % CG) * G1 * P:(g % CG + 1) * G1 * P]
                nc.scalar.activation(e_tiles[g][:], ct,
                                     mybir.ActivationFunctionType.Exp)
            for g in range(n_groups):
                ps = psum_s.tile([P, G1 * P], F32)
                nc.tensor.matmul(ps[:], ones_sb[:], e_tiles[g][:],
                                 start=True, stop=True)
                rv = rvpool.tile([P, G1 * P], F32)
                nc.vector.reciprocal_approx_fast(out=rv[:], in_=ps[:])
                nc.vector.tensor_tensor(w_tiles[g][:], e_tiles[g][:], rv[:],
                                        op=mybir.AluOpType.mult)

        # ---- Phase 2: weight-stationary block matmuls over the full batch ----
        ev = 0
        for gx in range(n_gx):
            xt = xpool.tile([P, XCHUNK * batch], BF16)
            nc.sync.dma_start(
                out=xt[:],
                in_=x[:, gx * XCHUNK * batch:(gx + 1) * XCHUNK * batch],
            )
            for jo in range(XCHUNK // OCHUNK):
                ot = opool.tile([P, OCHUNK * batch], BF16)
                for j2 in range(OCHUNK):
                    j = jo * OCHUNK + j2
                    n = gx * XCHUNK + j
                    for h in range(batch // PS_COLS):
                        pso = psum_o.tile([P, PS_COLS], F32)
                        for k in range(PS_COLS // NMM):
                            col = h * PS_COLS + k * NMM
                            nc.tensor.matmul(
                                pso[:, k * NMM:(k + 1) * NMM],
                                w_slice(n),
                                xt[:, j * batch + col:j * batch + col + NMM],
                                start=True,
                                stop=True,
                            )
                        evict_engine(ev)(
                            ot[:, j2 * batch + h * PS_COLS:
                               j2 * batch + (h + 1) * PS_COLS],
                            pso[:],
                        )
                        ev += 1
                nc.sync.dma_start(
                    out=out[:, (gx * XCHUNK + jo * OCHUNK) * batch:
                            (gx * XCHUNK + (jo + 1) * OCHUNK) * batch],
                    in_=ot[:],
                )


def build_program(batch=BATCH, blocks=BLOCKS_PER_CORE):
    nc = bacc.Bacc("TRN2", target_bir_lowering=False, debug=False)
    # x arrives host-repacked as m-major bf16 [m, n*b], see repack_x.
    x = nc.dram_tensor("x", [P, blocks * batch], BF16, kind="ExternalInput").ap()
    # c arrives host-repacked as m-major bf16 [m, n*o], see repack_c.
    c = nc.dram_tensor("c", [P, blocks * P], BF16, kind="ExternalInput").ap()
    # out leaves o-major bf16 [o, n*b], untransposed on host, see unpack_out.
    out = nc.dram_tensor("out", [P, blocks * batch], BF16, kind="ExternalOutput").ap()
    with tile.TileContext(nc) as tc:
        _body(tc, out, x, c, batch, blocks)
    nc.compile()
    return nc


_NC_CACHE = {}


def _get_nc():
    if "nc" not in _NC_CACHE:
        _NC_CACHE["nc"] = build_program()
    return _NC_CACHE["nc"]


def repack_c(c_shard):
    """[n, m, o] -> m-major bf16 [m, n*o]: one efficient DMA, half the bytes.

    bf16 c moves the end-to-end rel err from ~6e-3 to ~1.3e-2 (validated at
    full scale on the exact device arithmetic path) -- still 1.5x under the
    2e-2 budget -- and saves ~6.5 us of DMA."""
    n = c_shard.shape[0]
    return (
        c_shard.transpose(1, 0, 2)
        .astype(BF16_NP)
        .reshape(P, n * P)
    )


def repack_x(x_shard):
    """[b, n*m] fp32 -> m-major bf16 [m, n*b]: contraction dim on partitions."""
    batch, cols = x_shard.shape
    nb = cols // P
    return (
        x_shard.reshape(batch, nb, P)
        .transpose(2, 1, 0)
        .astype(BF16_NP)
        .reshape(P, nb * batch)
    )


def unpack_out(o_packed, batch, blocks):
    """o-major bf16 [o, n*b] -> [b, n*o] fp32."""
    return (
        np.asarray(o_packed)
        .reshape(P, blocks, batch)
        .transpose(2, 1, 0)
        .astype(np.float32)
        .reshape(batch, blocks * P)
    )


def _make_in_maps(x, c):
    xr = x.reshape(BATCH, N_CORES, XCOLS)
    in_maps = []
    for i in range(N_CORES):
        in_maps.append(
            {
                "x": repack_x(np.ascontiguousarray(xr[:, i, :])),
                "c": repack_c(c[i * BLOCKS_PER_CORE:(i + 1) * BLOCKS_PER_CORE]),
            }
        )
    return in_maps


def run_on_hw(x, c, trace=False):
    """Run the SPMD kernel on the 8 cores; returns (out, BassKernelResults)."""
    x = np.asarray(x, dtype=np.float32)
    c = np.asarray(c, dtype=np.float32)
    assert x.shape == (BATCH, LAYER), x.shape
    assert c.shape == (N_BLOCKS_TOTAL, P, P), c.shape
    nc = _get_nc()
    in_maps = _make_in_maps(x, c)
    res = None
    for attempt in range(3):
        try:
            res = run_bass_kernel_spmd(
                nc, in_maps, core_ids=list(range(N_CORES)), trace=trace
            )
            break
        except Exception:
            # Transient runtime failures (e.g. a device flake) are rare but
            # fatal to a single attempt; retry with a fresh dispatch.
            if attempt == 2:
                raise
    assert res is not None
    out = np.empty((BATCH, LAYER), dtype=np.float32)
    orv = out.reshape(BATCH, N_CORES, XCOLS)
    for i in range(N_CORES):
        orv[:, i, :] = unpack_out(res.results[i]["out"], BATCH, BLOCKS_PER_CORE)
    return out, res


def kernel(x, c):
    out, _ = run_on_hw(x, c, trace=False)
    return out


# revision 11
# speedup vs baseline: 2.5814x; 1.0574x over previous
"""Block-diagonal matmul with softmax-normalized weights, SPMD on 8 NeuronCores.

Computes: out[b, n*128+o] = sum_m x[b, n*128+m] * softmax(c[n], axis=m)[m, o]
for n in 512 independent 128x128 blocks, b in 2048 batch rows.

Sharding: blocks are fully independent -> shard the n_blocks axis across the
8 cores (64 blocks per core). Each core sees x columns [i*8192, (i+1)*8192),
blocks c[i*64:(i+1)*64], and produces the matching output column slice.

Layout + dtype strategy (rel-err budget is 2e-2; this path lands at ~1.4e-2,
validated at full scale on the exact arithmetic chain):
  - x is host-repacked per core to m-major bf16 [m=128, n*b] so the
    contraction dim m sits on SBUF partitions directly -- no PE transposes at
    all (they cost as much PE time as the matmuls themselves in the fp32
    version) -- and DMA traffic is halved vs fp32.
  - c is host-repacked to m-major bf16 [m=128, n*o] (one 2 MiB DMA with
    16 KiB per-partition rows).
  - Output is produced transposed ([o=128, n*b] bf16) straight from the
    matmul's natural PSUM orientation, DMA'd out in bf16 (halving write
    traffic), and untransposed/upcast on the host.

Per-core kernel (Tile framework). The run is DMA-bound (~66 MiB at the
~320-340 GB/s effective mixed-read/write HBM rate per core -> ~200 us), so the
whole design keeps the eviction engines (VectorE + ScalarE) and the DMA queues
free of anything serializing:
  - Softmax normalization never touches the weights: the matmul uses the
    UNNORMALIZED e = exp(c) (bf16, straight out of ScalarE, one table load) as
    the stationary operand, and the 1/colsum correction is folded into the
    PSUM eviction as a free per-partition scalar multiply (the block-n output
    sits in PSUM as [o, b], and 1/colsum[n, o] is constant along b). Earlier
    versions that normalized w up front serialized a 23 us reciprocal/multiply
    chain at the head of the VectorE queue, head-of-line blocking all PSUM
    evictions behind it and starving the DMA stream for ~25 us.
  - colsum(e_n) comes from a tiny N=1 matmul e_n^T @ ones per block, batched
    per 4-block group into one PSUM bank; one ~100 ns DVE reciprocal
    (Newton-Raphson approx, ~18 correct bits, colsums are ~128*E[exp] so no
    edge cases) turns each group's [o, 4] sums into scales. These are emitted
    lazily at group boundaries inside the phase-2 loop so they never dam up
    the PE/DVE queues.
  - Phase 2 is weight-stationary: for each block n, e_n [m,o] is the 128x128
    stationary operand and the whole 2048-col batch of xT_n [m,b] streams
    through as 4 N=512 bf16 matmuls into PSUM [o,b]. Evictions (PSUM fp32 ->
    SBUF bf16 with the scale) run at ~1.3 us per 2-bank half-block and total
    ~170 us -- more than one engine's worth -- so they alternate between
    VectorE and ScalarE (the first few stay on VectorE while ScalarE finishes
    the 16 Exps; ScalarE ops are issued Exp-batched because every activation
    function switch costs a 1.3 us ACT_TABLE_LOAD).
  - 4 MiB DMAs stream x in and 2 MiB DMAs stream results out (long
    per-partition bursts, few ~1 us dispatches, all on the sync queue; the c
    DMA dispatches on the scalar queue so it does not delay the first x).
"""

import numpy as np
from contextlib import ExitStack

import ml_dtypes

import concourse.bacc as bacc
import concourse.tile as tile
from concourse import mybir
from concourse.bass_utils import run_bass_kernel_spmd

F32 = mybir.dt.float32
BF16 = mybir.dt.bfloat16
BF16_NP = ml_dtypes.bfloat16
P = 128
N_CORES = 8
N_BLOCKS_TOTAL = 512
BLOCKS_PER_CORE = N_BLOCKS_TOTAL // N_CORES  # 64
BATCH = 2048
XCOLS = BLOCKS_PER_CORE * P  # 8192
LAYER = N_BLOCKS_TOTAL * P   # 65536


def _body(tc, out, x, c, batch, blocks):
    nc = tc.nc
    G1 = 4                         # blocks per sum group (one PSUM bank)
    XCHUNK = min(8, blocks)        # blocks per x DMA (4 MiB bf16)
    OCHUNK = min(4, blocks)        # blocks per out DMA (2 MiB bf16)
    NMM = min(512, batch)          # moving cols per matmul (one fp32 bank)
    PS_COLS = min(1024, batch)     # psum tile cols (two banks)
    n_gx = blocks // XCHUNK
    n_groups = blocks // G1
    n_evict = blocks * (batch // PS_COLS)
    # Early evictions stay on DVE while ScalarE finishes the Exps; after that,
    # alternate so the eviction work splits across both engines.
    ev_dve_only = max(4, n_evict // 12)

    def evict(i, out_ap, psum_ap, scale_ap):
        """PSUM fp32 -> SBUF bf16 with the softmax normalization folded in as
        a per-partition scalar multiply."""
        if i < ev_dve_only or i % 2 == 0:
            nc.vector.tensor_scalar_mul(out_ap, psum_ap, scale_ap)
        else:
            nc.scalar.mul(out_ap, psum_ap, scale_ap)

    with ExitStack() as ctx:
        const = ctx.enter_context(tc.tile_pool(name="const", bufs=1))
        ones_sb = const.tile([P, P], BF16)
        nc.vector.memset(ones_sb[:], 1.0)
        # Unnormalized weights e = exp(c), one tile per 4-block group.
        wpool = ctx.enter_context(tc.tile_pool(name="wpool", bufs=1))
        w_tiles = [wpool.tile([P, G1 * P], BF16, name=f"w{g}", tag=f"w{g}")
                   for g in range(n_groups)]
        # Per-group eviction scales rv[o, r] = 1/colsum(block g*4+r, o).
        rvpool = ctx.enter_context(tc.tile_pool(name="rvpool", bufs=1))
        rv_tiles = [rvpool.tile([P, G1], F32, name=f"rv{g}", tag=f"rv{g}")
                    for g in range(n_groups)]

        def w_slice(n):
            """AP for block n's unnormalized weights [m, o]."""
            g, r = divmod(n, G1)
            return w_tiles[g][:, r * P:(r + 1) * P]

        xpool = ctx.enter_context(tc.tile_pool(name="xpool", bufs=3))
        opool = ctx.enter_context(tc.tile_pool(name="opool", bufs=3))
        psum_o = ctx.enter_context(tc.tile_pool(name="psum_o", bufs=3, space="PSUM"))
        psum_s = ctx.enter_context(tc.tile_pool(name="psum_s", bufs=2, space="PSUM"))
        cpool = ctx.enter_context(tc.tile_pool(name="cpool", bufs=1))

        # ---- Phase 1: e = exp(c), Exp-batched on ScalarE ----
        ct = cpool.tile([P, blocks * P], BF16)
        # Scalar-queue HWDGE: the c dispatch runs concurrently with the x
        # dispatches on the sync queue instead of ahead of them.
        nc.scalar.dma_start(out=ct[:], in_=c[:])
        for g in range(n_groups):
            nc.scalar.activation(w_tiles[g][:],
                                 ct[:, g * G1 * P:(g + 1) * G1 * P],
                                 mybir.ActivationFunctionType.Exp)

        # ---- Phase 2: weight-stationary block matmuls over the full batch,
        # with the per-group colsum/reciprocal emitted lazily at group
        # boundaries so no engine queue is dammed up at the start.
        ev = 0
        groups_done = set()
        for gx in range(n_gx):
            xt = xpool.tile([P, XCHUNK * batch], BF16)
            nc.sync.dma_start(
                out=xt[:],
                in_=x[:, gx * XCHUNK * batch:(gx + 1) * XCHUNK * batch],
            )
            for jo in range(XCHUNK // OCHUNK):
                ot = opool.tile([P, OCHUNK * batch], BF16)
                for j2 in range(OCHUNK):
                    j = jo * OCHUNK + j2
                    n = gx * XCHUNK + j
                    g, r = divmod(n, G1)
                    if g not in groups_done:
                        groups_done.add(g)
                        ps = psum_s.tile([P, G1], F32)
                        for rr in range(G1):
                            nc.tensor.matmul(
                                ps[:, rr:rr + 1],
                                w_slice(g * G1 + rr),
                                ones_sb[:, 0:1],
                                start=True,
                                stop=True,
                            )
                        nc.vector.reciprocal_approx_fast(
                            out=rv_tiles[g][:], in_=ps[:])
                    for h in range(batch // PS_COLS):
                        pso = psum_o.tile([P, PS_COLS], F32)
                        for k in range(PS_COLS // NMM):
                            col = h * PS_COLS + k * NMM
                            nc.tensor.matmul(
                                pso[:, k * NMM:(k + 1) * NMM],
                                w_slice(n),
                                xt[:, j * batch + col:j * batch + col + NMM],
                                start=True,
                                stop=True,
                            )
                        evict(
                            ev,
                            ot[:, j2 * batch + h * PS_COLS:
                               j2 * batch + (h + 1) * PS_COLS],
                            pso[:],
                            rv_tiles[g][:, r:r + 1],
                        )
                        ev += 1
                nc.sync.dma_start(
                    out=out[:, (gx * XCHUNK + jo * OCHUNK) * batch:
                            (gx * XCHUNK + (jo + 1) * OCHUNK) * batch],
                    in_=ot[:],
                )


def build_program(batch=BATCH, blocks=BLOCKS_PER_CORE):
    nc = bacc.Bacc("TRN2", target_bir_lowering=False, debug=False)
    # x arrives host-repacked as m-major bf16 [m, n*b], see repack_x.
    x = nc.dram_tensor("x", [P, blocks * batch], BF16, kind="ExternalInput").ap()
    # c arrives host-repacked as m-major bf16 [m, n*o], see repack_c.
    c = nc.dram_tensor("c", [P, blocks * P], BF16, kind="ExternalInput").ap()
    # out leaves o-major bf16 [o, n*b], untransposed on host, see unpack_out.
    out = nc.dram_tensor("out", [P, blocks * batch], BF16, kind="ExternalOutput").ap()
    with tile.TileContext(nc) as tc:
        _body(tc, out, x, c, batch, blocks)
    nc.compile()
    return nc


_NC_CACHE = {}


def _get_nc():
    if "nc" not in _NC_CACHE:
        _NC_CACHE["nc"] = build_program()
    return _NC_CACHE["nc"]


def repack_c(c_shard):
    """[n, m, o] -> m-major bf16 [m, n*o]: one efficient DMA, half the bytes.

    bf16 c moves the end-to-end rel err from ~6e-3 to ~1.4e-2 (validated at
    full scale on the exact device arithmetic path) -- still 1.4x under the
    2e-2 budget -- and saves ~6.5 us of DMA."""
    n = c_shard.shape[0]
    return (
        c_shard.transpose(1, 0, 2)
        .astype(BF16_NP)
        .reshape(P, n * P)
    )


def repack_x(x_shard):
    """[b, n*m] fp32 -> m-major bf16 [m, n*b]: contraction dim on partitions."""
    batch, cols = x_shard.shape
    nb = cols // P
    return (
        x_shard.reshape(batch, nb, P)
        .transpose(2, 1, 0)
        .astype(BF16_NP)
        .reshape(P, nb * batch)
    )


def unpack_out(o_packed, batch, blocks):
    """o-major bf16 [o, n*b] -> [b, n*o] fp32."""
    return (
        np.asarray(o_packed)
        .reshape(P, blocks, batch)
        .transpose(2, 1, 0)
        .astype(np.float32)
        .reshape(batch, blocks * P)
    )


def _make_in_maps(x, c):
    xr = x.reshape(BATCH, N_CORES, XCOLS)
    in_maps = []
    for i in range(N_CORES):
        in_maps.append(
            {
                "x": repack_x(np.ascontiguousarray(xr[:, i, :])),
                "c": repack_c(c[i * BLOCKS_PER_CORE:(i + 1) * BLOCKS_PER_CORE]),
            }
        )
    return in_maps


def run_on_hw(x, c, trace=False):
    """Run the SPMD kernel on the 8 cores; returns (out, BassKernelResults)."""
    x = np.asarray(x, dtype=np.float32)
    c = np.asarray(c, dtype=np.float32)
    assert x.shape == (BATCH, LAYER), x.shape
    assert c.shape == (N_BLOCKS_TOTAL, P, P), c.shape
    nc = _get_nc()
    in_maps = _make_in_maps(x, c)
    res = None
    for attempt in range(3):
        try:
            res = run_bass_kernel_spmd(
                nc, in_maps, core_ids=list(range(N_CORES)), trace=trace
            )
            break
        except Exception:
            # Transient runtime failures (e.g. a device flake) are rare but
            # fatal to a single attempt; retry with a fresh dispatch.
            if attempt == 2:
                raise
    assert res is not None
    out = np.empty((BATCH, LAYER), dtype=np.float32)
    orv = out.reshape(BATCH, N_CORES, XCOLS)
    for i in range(N_CORES):
        orv[:, i, :] = unpack_out(res.results[i]["out"], BATCH, BLOCKS_PER_CORE)
    return out, res


def kernel(x, c):
    out, _ = run_on_hw(x, c, trace=False)
    return out


# revision 14
# speedup vs baseline: 2.6829x; 1.0393x over previous
"""Block-diagonal matmul with softmax-normalized weights, SPMD on 8 NeuronCores.

Computes: out[b, n*128+o] = sum_m x[b, n*128+m] * softmax(c[n], axis=m)[m, o]
for n in 512 independent 128x128 blocks, b in 2048 batch rows.

Sharding: blocks are fully independent -> shard the n_blocks axis across the
8 cores (64 blocks per core). Each core sees x columns [i*8192, (i+1)*8192),
blocks c[i*64:(i+1)*64], and produces the matching output column slice.

Layout + dtype strategy (rel-err budget is 2e-2; this path lands at ~1.4e-2,
validated at full scale on the exact arithmetic chain):
  - x is host-repacked per core to m-major bf16 [m=128, n*b] so the
    contraction dim m sits on SBUF partitions directly -- no PE transposes at
    all (they cost as much PE time as the matmuls themselves in the fp32
    version) -- and DMA traffic is halved vs fp32.
  - c is host-repacked to m-major bf16 [m=128, n*o] (one 2 MiB DMA with
    16 KiB per-partition rows).
  - Output is produced transposed ([o=128, n*b] bf16) straight from the
    matmul's natural PSUM orientation, DMA'd out in bf16 (halving write
    traffic), and untransposed/upcast on the host.

Per-core kernel (Tile framework). The run is DMA-bound (~66 MiB at the
~320-340 GB/s effective mixed-read/write HBM rate per core -> ~200 us), so the
whole design keeps the eviction engines (VectorE + ScalarE) and the DMA queues
free of anything serializing:
  - Softmax normalization never touches the weights: the matmul uses the
    UNNORMALIZED e = exp(c) (bf16, straight out of ScalarE, one table load) as
    the stationary operand, and the 1/colsum correction is folded into the
    PSUM eviction as a free per-partition scalar multiply (the block-n output
    sits in PSUM as [o, b], and 1/colsum[n, o] is constant along b). Earlier
    versions that normalized w up front serialized a 23 us reciprocal/multiply
    chain at the head of the VectorE queue, head-of-line blocking all PSUM
    evictions behind it and starving the DMA stream for ~25 us.
  - colsum(e_n) comes from a tiny N=1 matmul e_n^T @ ones per block, batched
    per 4-block group into one PSUM bank; one ~100 ns DVE reciprocal
    (Newton-Raphson approx, ~18 correct bits, colsums are ~128*E[exp] so no
    edge cases) turns each group's [o, 4] sums into scales. These are emitted
    lazily at group boundaries inside the phase-2 loop so they never dam up
    the PE/DVE queues.
  - Phase 2 is weight-stationary: for each block n, e_n [m,o] is the 128x128
    stationary operand and the whole 2048-col batch of xT_n [m,b] streams
    through as 4 N=512 bf16 matmuls into PSUM [o,b]. Evictions (PSUM fp32 ->
    SBUF bf16 with the scale) run at ~1.3 us per 2-bank half-block and total
    ~170 us -- more than one engine's worth -- so they alternate between
    VectorE and ScalarE (the first few stay on VectorE while ScalarE finishes
    the 16 Exps; ScalarE ops are issued Exp-batched because every activation
    function switch costs a 1.3 us ACT_TABLE_LOAD).
  - 4 MiB DMAs stream x in and 2 MiB DMAs stream results out (long
    per-partition bursts, few ~1 us dispatches, all on the sync queue; the c
    DMA dispatches on the scalar queue so it does not delay the first x).
"""

import numpy as np
from contextlib import ExitStack

import ml_dtypes

import concourse.bacc as bacc
import concourse.tile as tile
from concourse import mybir
from concourse.bass_utils import run_bass_kernel_spmd

F32 = mybir.dt.float32
BF16 = mybir.dt.bfloat16
BF16_NP = ml_dtypes.bfloat16
P = 128
N_CORES = 8
N_BLOCKS_TOTAL = 512
BLOCKS_PER_CORE = N_BLOCKS_TOTAL // N_CORES  # 64
BATCH = 2048
XCOLS = BLOCKS_PER_CORE * P  # 8192
LAYER = N_BLOCKS_TOTAL * P   # 65536


def _body(tc, out, x, c, batch, blocks):
    nc = tc.nc
    G1 = 4                         # blocks per sum group (one PSUM bank)
    OCHUNK = min(2, blocks)        # blocks per out DMA (1 MiB bf16)
    NMM = min(512, batch)          # moving cols per matmul (one fp32 bank)
    PS_COLS = min(1024, batch)     # psum tile cols (two banks)
    n_groups = blocks // G1
    n_evict = blocks * (batch // PS_COLS)
    # Early evictions stay on DVE while ScalarE finishes the Exps; after that,
    # alternate so the eviction work splits across both engines.
    ev_dve_only = max(4, n_evict // 8)
    # x DMA ramp: small chunks first so the matmul/eviction/out pipeline
    # starts ~15 us earlier than a uniform 8-block chunking would allow,
    # then 4 MiB chunks for long HBM read bursts.
    if blocks >= 8:
        x_chunks = [2, 2, 4] + [8] * ((blocks - 8) // 8)
    else:
        x_chunks = [blocks]
    assert sum(x_chunks) == blocks

    def evict(i, out_ap, psum_ap, scale_ap):
        """PSUM fp32 -> SBUF bf16 with the softmax normalization folded in as
        a per-partition scalar multiply."""
        if i < ev_dve_only or i % 2 == 0:
            nc.vector.tensor_scalar_mul(out_ap, psum_ap, scale_ap)
        else:
            nc.scalar.mul(out_ap, psum_ap, scale_ap)

    with ExitStack() as ctx:
        const = ctx.enter_context(tc.tile_pool(name="const", bufs=1))
        ones_sb = const.tile([P, P], BF16)
        nc.vector.memset(ones_sb[:], 1.0)
        # Unnormalized weights e = exp(c), one tile per 4-block group.
        wpool = ctx.enter_context(tc.tile_pool(name="wpool", bufs=1))
        w_tiles = [wpool.tile([P, G1 * P], BF16, name=f"w{g}", tag=f"w{g}")
                   for g in range(n_groups)]
        # Per-group eviction scales rv[o, r] = 1/colsum(block g*4+r, o).
        rvpool = ctx.enter_context(tc.tile_pool(name="rvpool", bufs=1))
        rv_tiles = [rvpool.tile([P, G1], F32, name=f"rv{g}", tag=f"rv{g}")
                    for g in range(n_groups)]

        def w_slice(n):
            """AP for block n's unnormalized weights [m, o]."""
            g, r = divmod(n, G1)
            return w_tiles[g][:, r * P:(r + 1) * P]

        xpools = {}
        for sz, bufs in ((2, 2), (4, 1), (8, 3)):
            if sz in x_chunks:
                xpools[sz] = ctx.enter_context(
                    tc.tile_pool(name=f"xpool{sz}", bufs=bufs))
        opool = ctx.enter_context(tc.tile_pool(name="opool", bufs=4))
        psum_o = ctx.enter_context(tc.tile_pool(name="psum_o", bufs=3, space="PSUM"))
        psum_s = ctx.enter_context(tc.tile_pool(name="psum_s", bufs=2, space="PSUM"))
        cpool = ctx.enter_context(tc.tile_pool(name="cpool", bufs=1))

        # ---- Phase 1: e = exp(c), Exp-batched on ScalarE ----
        # The c DMA is split into pieces so the first Exps run as soon as the
        # first piece lands (a single 2 MiB c DMA interleaves with the x
        # stream at descriptor granularity and only completes at ~24 us,
        # which delayed the entire pipeline). Scalar-queue HWDGE: dispatches
        # run concurrently with the x dispatches on the sync queue.
        CPIECE = max(1, n_groups // 4)  # groups per c piece
        c_tiles = []
        for p0 in range(0, n_groups, CPIECE):
            ct = cpool.tile([P, CPIECE * G1 * P], BF16, name=f"c{p0}",
                            tag=f"c{p0}")
            nc.scalar.dma_start(
                out=ct[:],
                in_=c[:, p0 * G1 * P:(p0 + CPIECE) * G1 * P],
            )
            c_tiles.append(ct)
            for g in range(p0, p0 + CPIECE):
                nc.scalar.activation(
                    w_tiles[g][:],
                    ct[:, (g - p0) * G1 * P:(g - p0 + 1) * G1 * P],
                    mybir.ActivationFunctionType.Exp)

        # ---- Phase 2: weight-stationary block matmuls over the full batch,
        # with the per-group colsum/reciprocal emitted lazily at group
        # boundaries so no engine queue is dammed up at the start.
        ev = 0
        groups_done = set()
        nb0 = 0
        for XCHUNK in x_chunks:
            gx0 = nb0
            xt = xpools[XCHUNK].tile([P, XCHUNK * batch], BF16)
            nc.sync.dma_start(
                out=xt[:],
                in_=x[:, gx0 * batch:(gx0 + XCHUNK) * batch],
            )
            nb0 += XCHUNK
            for jo in range(XCHUNK // OCHUNK):
                ot = opool.tile([P, OCHUNK * batch], BF16)
                for j2 in range(OCHUNK):
                    j = jo * OCHUNK + j2
                    n = gx0 + j
                    g, r = divmod(n, G1)
                    if g not in groups_done:
                        groups_done.add(g)
                        ps = psum_s.tile([P, G1], F32)
                        for rr in range(G1):
                            nc.tensor.matmul(
                                ps[:, rr:rr + 1],
                                w_slice(g * G1 + rr),
                                ones_sb[:, 0:1],
                                start=True,
                                stop=True,
                            )
                        nc.vector.reciprocal_approx_fast(
                            out=rv_tiles[g][:], in_=ps[:])
                    for h in range(batch // PS_COLS):
                        pso = psum_o.tile([P, PS_COLS], F32)
                        for k in range(PS_COLS // NMM):
                            col = h * PS_COLS + k * NMM
                            nc.tensor.matmul(
                                pso[:, k * NMM:(k + 1) * NMM],
                                w_slice(n),
                                xt[:, j * batch + col:j * batch + col + NMM],
                                start=True,
                                stop=True,
                            )
                        evict(
                            ev,
                            ot[:, j2 * batch + h * PS_COLS:
                               j2 * batch + (h + 1) * PS_COLS],
                            pso[:],
                            rv_tiles[g][:, r:r + 1],
                        )
                        ev += 1
                nc.sync.dma_start(
                    out=out[:, (gx0 + jo * OCHUNK) * batch:
                            (gx0 + (jo + 1) * OCHUNK) * batch],
                    in_=ot[:],
                )


def build_program(batch=BATCH, blocks=BLOCKS_PER_CORE):
    nc = bacc.Bacc("TRN2", target_bir_lowering=False, debug=False)
    # x arrives host-repacked as m-major bf16 [m, n*b], see repack_x.
    x = nc.dram_tensor("x", [P, blocks * batch], BF16, kind="ExternalInput").ap()
    # c arrives host-repacked as m-major bf16 [m, n*o], see repack_c.
    c = nc.dram_tensor("c", [P, blocks * P], BF16, kind="ExternalInput").ap()
    # out leaves o-major bf16 [o, n*b], untransposed on host, see unpack_out.
    out = nc.dram_tensor("out", [P, blocks * batch], BF16, kind="ExternalOutput").ap()
    with tile.TileContext(nc) as tc:
        _body(tc, out, x, c, batch, blocks)
    nc.compile()
    return nc


_NC_CACHE = {}


def _get_nc():
    if "nc" not in _NC_CACHE:
        _NC_CACHE["nc"] = build_program()
    return _NC_CACHE["nc"]


def repack_c(c_shard):
    """[n, m, o] -> m-major bf16 [m, n*o]: one efficient DMA, half the bytes.

    bf16 c moves the end-to-end rel err from ~6e-3 to ~1.4e-2 (validated at
    full scale on the exact device arithmetic path) -- still 1.4x under the
    2e-2 budget -- and saves ~6.5 us of DMA."""
    n = c_shard.shape[0]
    return (
        c_shard.transpose(1, 0, 2)
        .astype(BF16_NP)
        .reshape(P, n * P)
    )


def repack_x(x_shard):
    """[b, n*m] fp32 -> m-major bf16 [m, n*b]: contraction dim on partitions."""
    batch, cols = x_shard.shape
    nb = cols // P
    return (
        x_shard.reshape(batch, nb, P)
        .transpose(2, 1, 0)
        .astype(BF16_NP)
        .reshape(P, nb * batch)
    )


def unpack_out(o_packed, batch, blocks):
    """o-major bf16 [o, n*b] -> [b, n*o] fp32."""
    return (
        np.asarray(o_packed)
        .reshape(P, blocks, batch)
        .transpose(2, 1, 0)
        .astype(np.float32)
        .reshape(batch, blocks * P)
    )


def _make_in_maps(x, c):
    xr = x.reshape(BATCH, N_CORES, XCOLS)
    in_maps = []
    for i in range(N_CORES):
        in_maps.append(
            {
                "x": repack_x(np.ascontiguousarray(xr[:, i, :])),
                "c": repack_c(c[i * BLOCKS_PER_CORE:(i + 1) * BLOCKS_PER_CORE]),
            }
        )
    return in_maps


def run_on_hw(x, c, trace=False):
    """Run the SPMD kernel on the 8 cores; returns (out, BassKernelResults)."""
    x = np.asarray(x, dtype=np.float32)
    c = np.asarray(c, dtype=np.float32)
    assert x.shape == (BATCH, LAYER), x.shape
    assert c.shape == (N_BLOCKS_TOTAL, P, P), c.shape
    nc = _get_nc()
    in_maps = _make_in_maps(x, c)
    res = None
    for attempt in range(3):
        try:
            res = run_bass_kernel_spmd(
                nc, in_maps, core_ids=list(range(N_CORES)), trace=trace
            )
            break
        except Exception:
            # Transient runtime failures (e.g. a device flake) are rare but
            # fatal to a single attempt; retry with a fresh dispatch.
            if attempt == 2:
                raise
    assert res is not None
    out = np.empty((BATCH, LAYER), dtype=np.float32)
    orv = out.reshape(BATCH, N_CORES, XCOLS)
    for i in range(N_CORES):
        orv[:, i, :] = unpack_out(res.results[i]["out"], BATCH, BLOCKS_PER_CORE)
    return out, res


def kernel(x, c):
    out, _ = run_on_hw(x, c, trace=False)
    return out


# revision 20
# speedup vs baseline: 2.7392x; 1.0210x over previous
"""Block-diagonal matmul with softmax-normalized weights, SPMD on 8 NeuronCores.

Computes: out[b, n*128+o] = sum_m x[b, n*128+m] * softmax(c[n], axis=m)[m, o]
for n in 512 independent 128x128 blocks, b in 2048 batch rows.

Sharding: blocks are fully independent -> shard the n_blocks axis across the
8 cores (64 blocks per core). Each core sees x columns [i*8192, (i+1)*8192),
blocks c[i*64:(i+1)*64], and produces the matching output column slice.

Layout + dtype strategy (rel-err budget is 2e-2; this path lands at ~1.4e-2,
validated at full scale on the exact arithmetic chain):
  - x is host-repacked per core to m-major bf16 [m=128, n*b] so the
    contraction dim m sits on SBUF partitions directly -- no PE transposes at
    all (they cost as much PE time as the matmuls themselves in the fp32
    version) -- and DMA traffic is halved vs fp32.
  - c is host-repacked to m-major bf16 [m=128, n*o] (one 2 MiB DMA with
    16 KiB per-partition rows).
  - Output is produced transposed ([o=128, n*b] bf16) straight from the
    matmul's natural PSUM orientation, DMA'd out in bf16 (halving write
    traffic), and untransposed/upcast on the host.

Per-core kernel (Tile framework). The run is DMA-bound (~66 MiB at the
~320-340 GB/s effective mixed-read/write HBM rate per core -> ~200 us), so the
whole design keeps the eviction engines (VectorE + ScalarE) and the DMA queues
free of anything serializing:
  - Softmax normalization never touches the weights: the matmul uses the
    UNNORMALIZED e = exp(c) (bf16, straight out of ScalarE, one table load) as
    the stationary operand, and the 1/colsum correction is folded into the
    PSUM eviction as a free per-partition scalar multiply (the block-n output
    sits in PSUM as [o, b], and 1/colsum[n, o] is constant along b). Earlier
    versions that normalized w up front serialized a 23 us reciprocal/multiply
    chain at the head of the VectorE queue, head-of-line blocking all PSUM
    evictions behind it and starving the DMA stream for ~25 us.
  - colsum(e_n) comes from a tiny N=1 matmul e_n^T @ ones per block, batched
    per 4-block group into one PSUM bank; one ~100 ns DVE reciprocal
    (Newton-Raphson approx, ~18 correct bits, colsums are ~128*E[exp] so no
    edge cases) turns each group's [o, 4] sums into scales. These are emitted
    lazily at group boundaries inside the phase-2 loop so they never dam up
    the PE/DVE queues.
  - Phase 2 is weight-stationary: for each block n, e_n [m,o] is the 128x128
    stationary operand and the whole 2048-col batch of xT_n [m,b] streams
    through as 4 N=512 bf16 matmuls into PSUM [o,b]. Evictions (PSUM fp32 ->
    SBUF bf16 with the scale) run at ~1.3 us per 2-bank half-block and total
    ~170 us -- more than one engine's worth -- so they alternate between
    VectorE and ScalarE (the first few stay on VectorE while ScalarE finishes
    the 16 Exps; ScalarE ops are issued Exp-batched because every activation
    function switch costs a 1.3 us ACT_TABLE_LOAD).
  - 4 MiB DMAs stream x in and 2 MiB DMAs stream results out (long
    per-partition bursts, few ~1 us dispatches, all on the sync queue; the c
    DMA dispatches on the scalar queue so it does not delay the first x).
"""

import numpy as np
from contextlib import ExitStack

import ml_dtypes

import concourse.bacc as bacc
import concourse.tile as tile
from concourse import mybir
from concourse.bass_utils import run_bass_kernel_spmd

F32 = mybir.dt.float32
BF16 = mybir.dt.bfloat16
BF16_NP = ml_dtypes.bfloat16
P = 128
N_CORES = 8
N_BLOCKS_TOTAL = 512
BLOCKS_PER_CORE = N_BLOCKS_TOTAL // N_CORES  # 64
BATCH = 2048
XCOLS = BLOCKS_PER_CORE * P  # 8192
LAYER = N_BLOCKS_TOTAL * P   # 65536


def _body(tc, out, x, c, batch, blocks):
    nc = tc.nc
    G1 = 4                         # blocks per sum group (one PSUM bank)
    OCHUNK = min(2, blocks)        # blocks per out DMA (1 MiB bf16)
    NMM = min(512, batch)          # moving cols per matmul (one fp32 bank)
    PS_COLS = min(1024, batch)     # psum tile cols (two banks)
    n_groups = blocks // G1
    n_evict = blocks * (batch // PS_COLS)
    # Early evictions stay on DVE while ScalarE finishes the Exps; after that,
    # alternate so the eviction work splits across both engines.
    ev_dve_only = 6
    # x DMA ramp: small chunks at BOTH ends -- at the head so the
    # matmul/eviction/out pipeline starts ~15 us earlier than a uniform
    # 8-block chunking would allow, and at the tail so the final blocks'
    # matmuls are not gated on a whole 4 MiB chunk landing (the x->SBUF
    # dependency is tile-granular). 4 MiB chunks in the middle for long HBM
    # read bursts.
    if blocks >= 16:
        x_chunks = [2] * 4 + [8] * ((blocks - 16) // 8) + [2] * 4
    elif blocks >= 8:
        x_chunks = [2] * (blocks // 2)
    else:
        x_chunks = [blocks]
    assert sum(x_chunks) == blocks

    def evict(i, out_ap, psum_ap, scale_ap):
        """PSUM fp32 -> SBUF bf16 with the softmax normalization folded in as
        a per-partition scalar multiply."""
        if i < ev_dve_only or i % 2 == 0:
            nc.vector.tensor_scalar_mul(out_ap, psum_ap, scale_ap)
        else:
            nc.scalar.mul(out_ap, psum_ap, scale_ap)

    with ExitStack() as ctx:
        const = ctx.enter_context(tc.tile_pool(name="const", bufs=1))
        ones_sb = const.tile([P, P], BF16)
        nc.vector.memset(ones_sb[:], 1.0)
        # Unnormalized weights e = exp(c), one tile per 4-block group.
        wpool = ctx.enter_context(tc.tile_pool(name="wpool", bufs=1))
        w_tiles = [wpool.tile([P, G1 * P], BF16, name=f"w{g}", tag=f"w{g}")
                   for g in range(n_groups)]
        # Per-group eviction scales rv[o, r] = 1/colsum(block g*4+r, o).
        rvpool = ctx.enter_context(tc.tile_pool(name="rvpool", bufs=1))
        rv_tiles = [rvpool.tile([P, G1], F32, name=f"rv{g}", tag=f"rv{g}")
                    for g in range(n_groups)]

        def w_slice(n):
            """AP for block n's unnormalized weights [m, o]."""
            g, r = divmod(n, G1)
            return w_tiles[g][:, r * P:(r + 1) * P]

        xpools = {}
        for sz, bufs in ((2, 2), (8, 4)):
            if sz in x_chunks:
                xpools[sz] = ctx.enter_context(
                    tc.tile_pool(name=f"xpool{sz}", bufs=bufs))
        opool = ctx.enter_context(tc.tile_pool(name="opool", bufs=4))
        psum_o = ctx.enter_context(tc.tile_pool(name="psum_o", bufs=3, space="PSUM"))
        psum_s = ctx.enter_context(tc.tile_pool(name="psum_s", bufs=2, space="PSUM"))
        cpool = ctx.enter_context(tc.tile_pool(name="cpool", bufs=2))

        # ---- Phase 1: e = exp(c), Exp-batched on ScalarE ----
        # The c DMA is split into pieces so the first Exps run as soon as the
        # first piece lands (a single 2 MiB c DMA interleaves with the x
        # stream at descriptor granularity and only completes at ~24 us,
        # which delayed the entire pipeline). Scalar-queue HWDGE: dispatches
        # run concurrently with the x dispatches on the sync queue.
        CPIECE = max(1, n_groups // 4)  # groups per c piece
        c_tiles = []
        # The pieces share one tag (2 rotating buffers): piece k reuses piece
        # k-2's buffer once its Exps have read it, saving 8 KiB of SBUF.
        for p0 in range(0, n_groups, CPIECE):
            ct = cpool.tile([P, CPIECE * G1 * P], BF16, name=f"c{p0}",
                            tag="cpiece")
            nc.scalar.dma_start(
                out=ct[:],
                in_=c[:, p0 * G1 * P:(p0 + CPIECE) * G1 * P],
            )
            c_tiles.append(ct)
            for g in range(p0, p0 + CPIECE):
                nc.scalar.activation(
                    w_tiles[g][:],
                    ct[:, (g - p0) * G1 * P:(g - p0 + 1) * G1 * P],
                    mybir.ActivationFunctionType.Exp)

        # ---- Phase 2: weight-stationary block matmuls over the full batch,
        # with the per-group colsum/reciprocal emitted lazily at group
        # boundaries so no engine queue is dammed up at the start.
        ev = 0
        groups_done = set()
        nb0 = 0
        for XCHUNK in x_chunks:
            gx0 = nb0
            xt = xpools[XCHUNK].tile([P, XCHUNK * batch], BF16)
            nc.sync.dma_start(
                out=xt[:],
                in_=x[:, gx0 * batch:(gx0 + XCHUNK) * batch],
            )
            nb0 += XCHUNK
            for jo in range(XCHUNK // OCHUNK):
                ot = opool.tile([P, OCHUNK * batch], BF16)
                for j2 in range(OCHUNK):
                    j = jo * OCHUNK + j2
                    n = gx0 + j
                    g, r = divmod(n, G1)
                    if g not in groups_done:
                        groups_done.add(g)
                        ps = psum_s.tile([P, G1], F32)
                        for rr in range(G1):
                            nc.tensor.matmul(
                                ps[:, rr:rr + 1],
                                w_slice(g * G1 + rr),
                                ones_sb[:, 0:1],
                                start=True,
                                stop=True,
                            )
                        nc.vector.reciprocal_approx_fast(
                            out=rv_tiles[g][:], in_=ps[:])
                    for h in range(batch // PS_COLS):
                        pso = psum_o.tile([P, PS_COLS], F32)
                        for k in range(PS_COLS // NMM):
                            col = h * PS_COLS + k * NMM
                            nc.tensor.matmul(
                                pso[:, k * NMM:(k + 1) * NMM],
                                w_slice(n),
                                xt[:, j * batch + col:j * batch + col + NMM],
                                start=True,
                                stop=True,
                            )
                        evict(
                            ev,
                            ot[:, j2 * batch + h * PS_COLS:
                               j2 * batch + (h + 1) * PS_COLS],
                            pso[:],
                            rv_tiles[g][:, r:r + 1],
                        )
                        ev += 1
                nc.sync.dma_start(
                    out=out[:, (gx0 + jo * OCHUNK) * batch:
                            (gx0 + (jo + 1) * OCHUNK) * batch],
                    in_=ot[:],
                )


def build_program(batch=BATCH, blocks=BLOCKS_PER_CORE):
    nc = bacc.Bacc("TRN2", target_bir_lowering=False, debug=False)
    # x arrives host-repacked as m-major bf16 [m, n*b], see repack_x.
    x = nc.dram_tensor("x", [P, blocks * batch], BF16, kind="ExternalInput").ap()
    # c arrives host-repacked as m-major bf16 [m, n*o], see repack_c.
    c = nc.dram_tensor("c", [P, blocks * P], BF16, kind="ExternalInput").ap()
    # out leaves o-major bf16 [o, n*b], untransposed on host, see unpack_out.
    out = nc.dram_tensor("out", [P, blocks * batch], BF16, kind="ExternalOutput").ap()
    with tile.TileContext(nc) as tc:
        _body(tc, out, x, c, batch, blocks)
    nc.compile()
    return nc


_NC_CACHE = {}


def _get_nc():
    if "nc" not in _NC_CACHE:
        _NC_CACHE["nc"] = build_program()
    return _NC_CACHE["nc"]


def repack_c(c_shard):
    """[n, m, o] -> m-major bf16 [m, n*o]: one efficient DMA, half the bytes.

    bf16 c moves the end-to-end rel err from ~6e-3 to ~1.4e-2 (validated at
    full scale on the exact device arithmetic path) -- still 1.4x under the
    2e-2 budget -- and saves ~6.5 us of DMA."""
    n = c_shard.shape[0]
    return (
        c_shard.transpose(1, 0, 2)
        .astype(BF16_NP)
        .reshape(P, n * P)
    )


def repack_x(x_shard):
    """[b, n*m] fp32 -> m-major bf16 [m, n*b]: contraction dim on partitions."""
    batch, cols = x_shard.shape
    nb = cols // P
    return (
        x_shard.reshape(batch, nb, P)
        .transpose(2, 1, 0)
        .astype(BF16_NP)
        .reshape(P, nb * batch)
    )


def unpack_out(o_packed, batch, blocks):
    """o-major bf16 [o, n*b] -> [b, n*o] fp32."""
    return (
        np.asarray(o_packed)
        .reshape(P, blocks, batch)
        .transpose(2, 1, 0)
        .astype(np.float32)
        .reshape(batch, blocks * P)
    )


def _make_in_maps(x, c):
    xr = x.reshape(BATCH, N_CORES, XCOLS)
    in_maps = []
    for i in range(N_CORES):
        in_maps.append(
            {
                "x": repack_x(np.ascontiguousarray(xr[:, i, :])),
                "c": repack_c(c[i * BLOCKS_PER_CORE:(i + 1) * BLOCKS_PER_CORE]),
            }
        )
    return in_maps


def run_on_hw(x, c, trace=False):
    """Run the SPMD kernel on the 8 cores; returns (out, BassKernelResults)."""
    x = np.asarray(x, dtype=np.float32)
    c = np.asarray(c, dtype=np.float32)
    assert x.shape == (BATCH, LAYER), x.shape
    assert c.shape == (N_BLOCKS_TOTAL, P, P), c.shape
    nc = _get_nc()
    in_maps = _make_in_maps(x, c)
    res = None
    for attempt in range(3):
        try:
            res = run_bass_kernel_spmd(
                nc, in_maps, core_ids=list(range(N_CORES)), trace=trace
            )
            break
        except Exception:
            # Transient runtime failures (e.g. a device flake) are rare but
            # fatal to a single attempt; retry with a fresh dispatch.
            if attempt == 2:
                raise
    assert res is not None
    out = np.empty((BATCH, LAYER), dtype=np.float32)
    orv = out.reshape(BATCH, N_CORES, XCOLS)
    for i in range(N_CORES):
        orv[:, i, :] = unpack_out(res.results[i]["out"], BATCH, BLOCKS_PER_CORE)
    return out, res


def kernel(x, c):
    out, _ = run_on_hw(x, c, trace=False)
    return out
